# revision 24
# baseline (speedup 1.0000x reference)
"""Coordinate-Attention kernel for Trainium2, data-parallel over batch on 8 NeuronCores.

Reference computation (per batch b):
  xh[c,h] = mean_w x[c,h,w]; xw[c,w] = mean_h x[c,h,w]
  y = hswish(BN(w1 @ concat(xh, xw) + b1))            # [Cm=128, 128]
  gh = sigmoid(w2 @ y[:, :64] + b2)                    # [256, 64]
  gw = sigmoid(w3 @ y[:, 64:] + b3)                    # [256, 64]
  out[c,h,w] = x[c,h,w] * gh[c,h] * gw[c,w]

Host folds BN into w1/b1 and the 1/64 pooling mean into w1. Each core
processes 4 batches; x is sharded on B across the 8 cores.

v6 ("0-depth split-gate"): wire stays bf16 (x and out cast on host; halves
HBM traffic).  A single pass is inherently serial around the DVE: the 16
gate multiplies are 2.2us each in 2x mode (HW-measured) and nothing else
can run them, so the schedule exists to keep the DVE queue dense from the
first reduce to the last mul:

  loads   all 8 x DMAs on the Sync ring (no sem waits; batch 0 split into
          half-height chunks so pooling starts ~2us earlier), while ~3.5us
          of throwaway matmuls warm the PE out of its cold 1.2 GHz pstate;
  per batch b:  pool A+B [PE] -> reduce+hswish per branch [DVE, right
          after its pass] -> gw gate then gh gate [PE matmul + ACT
          sigmoid; gw first because the first multiply consumes only gw]
          -> 2x-mode gate muls in place + stores [DVE + Act ring].

  The DVE queue is [sm(0), M(0), sm(1), M(1), ...]: pooling (7us) is
  shorter than a mul block (9us), so the PE stays a batch ahead and each
  batch's sigmoids land before the DVE reaches its muls.  Everything
  gate-chain-related runs on DVE/PE/ACT only -- gpsimd tensor ops measure
  ~2-3us each in-chain and serialized v2/v3 (80-86us vs 68us).

PSUM slots are bank-padded (8 banks): psA x3 + psB x3 + gate x2.
The gh (broadcast over w) multiply keeps the duplicated-pair gh2[c,h,2]
trick so every DVE operand's innermost AP dim is packed 2-wide (2x mode);
the last batch's muls+stores are h-quartered to shorten the final store
tail, and batch 0's loads arrive in quarter-height chunks so the first
pooling matmuls start as early as possible.
"""
import sys

for _p in ("/opt/trn_rl_repo",):
    if _p not in sys.path:
        sys.path.insert(0, _p)

import numpy as np

import concourse.bacc as bacc
import concourse.bass as bass
import concourse.tile as tile
import concourse.mybir as mybir
from concourse.bass_utils import run_bass_kernel_spmd

N_CORES = 8
B, C, H, W = 32, 256, 64, 64
B_LOC = B // N_CORES  # 4
CB = C // 128  # 2 channel blocks
F32 = mybir.dt.float32
BF16 = mybir.dt.bfloat16
NP_BF16 = mybir.dt.np(BF16)
AF = mybir.ActivationFunctionType
ALU = mybir.AluOpType
AX = mybir.AxisListType

_NC_CACHE = {}


def build_module(
    n_iter: int = 1,
    xs_bufs: int = 8,
    unroll: int = 1,
    no_mul: bool = False,    # timing-only: skip the big DVE gate multiplies
    no_gates: bool = False,  # timing-only: constant gates, skip gate compute
    no_pe: bool = False,     # timing-only: skip pooling passes
    decouple: bool = False,  # timing-only: compute gates but muls read consts
    sig_copy: bool = False,  # timing-only: ACT Copy instead of Sigmoid
    mul_reps: int = 1,       # timing-only: repeat each gate multiply N times
):
    """phase-separated wire-bf16 module. n_iter>1 wraps the workload in a
    hardware For_i loop (timing only; the graded path uses n_iter=1)."""
    nc = bacc.Bacc("TRN2", debug=False, num_devices=N_CORES)
    x_d = nc.dram_tensor("x", [B_LOC, C, H, W], BF16, kind="ExternalInput").ap()
    w1t_d = nc.dram_tensor("w1t", [128, CB, 128], BF16, kind="ExternalInput").ap()
    b1c_d = nc.dram_tensor("b1c", [128, 1], F32, kind="ExternalInput").ap()
    b1c2_d = nc.dram_tensor("b1c2", [128, 1], F32, kind="ExternalInput").ap()
    w2t_d = nc.dram_tensor("w2t", [128, 2, 128], BF16, kind="ExternalInput").ap()
    b2c_d = nc.dram_tensor("b2c", [128, 2], F32, kind="ExternalInput").ap()
    w3t_d = nc.dram_tensor("w3t", [128, 2, 128], BF16, kind="ExternalInput").ap()
    b3c_d = nc.dram_tensor("b3c", [128, 2], F32, kind="ExternalInput").ap()
    out_d = nc.dram_tensor("out", [B_LOC, C, H, W], BF16, kind="ExternalOutput").ap()

    NG = 8  # h (resp. w) rows folded per pooling matmul (512 columns)

    from contextlib import ExitStack

    with tile.TileContext(nc) as tc, ExitStack() as ctx:
        singles = ctx.enter_context(tc.tile_pool(name="singles", bufs=1))
        xs_pool = ctx.enter_context(tc.tile_pool(name="xs", bufs=xs_bufs))
        small_pool = ctx.enter_context(tc.tile_pool(name="small", bufs=4))
        ps_pool = ctx.enter_context(tc.tile_pool(name="ps", bufs=3, space="PSUM"))

        def load_weights():
            # weight DMAs on the Sync ring so the Act ring starts the
            # (critical) first-iteration x loads immediately
            w1t_sb = singles.tile([128, CB, 128], BF16, name="w1t_sb", tag="w1t_sb")
            nc.sync.dma_start(out=w1t_sb, in_=w1t_d)
            b1c_sb = singles.tile([128, 1], F32, name="b1c_sb", tag="b1c_sb")
            nc.sync.dma_start(out=b1c_sb, in_=b1c_d)
            b1c2_sb = singles.tile([128, 1], F32, name="b1c2_sb", tag="b1c2_sb")
            nc.sync.dma_start(out=b1c2_sb, in_=b1c2_d)
            w2t_sb = singles.tile([128, 2, 128], BF16, name="w2t_sb", tag="w2t_sb")
            nc.sync.dma_start(out=w2t_sb, in_=w2t_d)
            b2c_sb = singles.tile([128, 2], F32, name="b2c_sb", tag="b2c_sb")
            nc.sync.dma_start(out=b2c_sb, in_=b2c_d)
            w3t_sb = singles.tile([128, 2, 128], BF16, name="w3t_sb", tag="w3t_sb")
            nc.sync.dma_start(out=w3t_sb, in_=w3t_d)
            b3c_sb = singles.tile([128, 2], F32, name="b3c_sb", tag="b3c_sb")
            nc.sync.dma_start(out=b3c_sb, in_=b3c_d)
            return w1t_sb, b1c_sb, b1c2_sb, w2t_sb, b2c_sb, w3t_sb, b3c_sb

        def body(weights):
            w1t_sb, b1c_sb, b1c2_sb, w2t_sb, b2c_sb, w3t_sb, b3c_sb = weights

            # ---- phase L: all batch loads, Sync ring (loads alone: no sem
            # waits, so every iteration's loads dispatch the moment the SP
            # sequencer reaches them and the DMA engines always have load
            # work queued) ----
            # PE warm-up: ~3.5us of throwaway matmuls during the load ramp
            # so the PE_HAM clock gate is already at 2.4 GHz when the first
            # pooling matmul issues (cold pstate is 1.2 GHz).
            warm = singles.tile([128, 512], BF16, name="warm", tag="warm")
            nc.gpsimd.memset(warm, 0.0)
            wps = ps_pool.tile([128, 2, 64], F32, name="wps", tag="gp", bufs=2)
            for _ in range(14):
                nc.tensor.matmul(
                    wps[:, 0], lhsT=warm[:, :128], rhs=warm[:, :64],
                    start=True, stop=True,
                )

            xtf = []
            for b in range(B_LOC):
                t = xs_pool.tile([128, CB, H, W], BF16, name="xtf", tag="xtf")
                xtf.append(t)
                for cb in range(CB):
                    if b == 0:
                        # split the first batch into quarter-height chunks so
                        # pass A's first matmuls start as early as possible
                        QH = H // 4
                        for q in range(4):
                            nc.sync.dma_start(
                                out=t[:, cb, q * QH : (q + 1) * QH],
                                in_=x_d[
                                    b, cb * 128 : (cb + 1) * 128,
                                    q * QH : (q + 1) * QH,
                                ],
                            )
                    else:
                        nc.sync.dma_start(
                            out=t[:, cb], in_=x_d[b, cb * 128 : (cb + 1) * 128]
                        )

            # ---- per-batch interleaved pipeline ----
            # Emission per batch b:  pool(b) [PE] ; reduces+hswish(b) [DVE] ;
            # gate matmuls+sigmoids(b-1) [PE/ACT] ; muls+stores(b-1) [DVE/ACT].
            # DVE queue is [smalls(0), smalls(1), M(0), smalls(2), M(1), ...]
            # so the first mul block waits only on batch-0/1 pooling (not the
            # whole pooling phase), and every cross-engine link in the gate
            # chain gets a full batch period of slack.
            def pool(b):
                psA = ps_pool.tile([128, NG, W], F32, name="psA", tag="psA", bufs=3)
                psB = ps_pool.tile([128, H, NG], F32, name="psB", tag="psB", bufs=3)
                if no_pe:
                    return psA, psB
                # pass A: fold h by its low 3 bits; psA[m,i,w] accumulates
                # over cb and g with rhs a natural [c,8h,w] slice.
                for cb in range(CB):
                    for g in range(H // NG):
                        nc.tensor.matmul(
                            psA,
                            lhsT=w1t_sb[:, cb, :],
                            rhs=xtf[b][:, cb, g * NG : (g + 1) * NG, :],
                            start=(g == 0 and cb == 0),
                            stop=(g == H // NG - 1 and cb == CB - 1),
                        )
                # pass B: fold w by its low 3 bits; psB[m,h,j] accumulates
                # over g with rhs a natural [c,h,8w] slice (16B-contiguous
                # runs; a transposed-rhs view measures ~4x slower PE column
                # fetch on real HW).
                for cb in range(CB):
                    for g in range(W // NG):
                        nc.tensor.matmul(
                            psB,
                            lhsT=w1t_sb[:, cb, :],
                            rhs=xtf[b][:, cb, :, g * NG : (g + 1) * NG],
                            start=(g == 0 and cb == 0),
                            stop=(g == W // NG - 1 and cb == CB - 1),
                        )
                return psA, psB

            def hswish2(psA, psB):
                """Both branches fused: s[:,0]=reduce(psA^T) (w branch),
                s[:,1]=reduce(psB) (h branch), then ONE ts/clamp/stt pass
                over [128,2,64].  All on DVE (no cross-engine hops; the Act
                engine only ever runs Sigmoid, avoiding table reloads).
                t = clip(s/6 + (b1c/6+.5), 0, 1);  y = (s + b1c) * t."""
                s_t = small_pool.tile([128, 2, 64], F32, name="s_wh", tag="s_wh")
                nc.vector.reduce_sum(
                    out=s_t[:, 0], in_=psA.transpose([0, 2, 1]), axis=AX.X
                )
                nc.vector.reduce_sum(out=s_t[:, 1], in_=psB, axis=AX.X)
                t_t = small_pool.tile([128, 2, 64], F32, name="t_wh", tag="t_wh")
                nc.vector.tensor_scalar(
                    t_t, s_t, 1.0 / 6.0, b1c2_sb[:, 0:1], ALU.mult, ALU.add
                )
                nc.vector.tensor_scalar(t_t, t_t, 0.0, 1.0, ALU.max, ALU.min)
                y_t = small_pool.tile([128, 2, 64], BF16, name="y_wh", tag="y_wh")
                nc.vector.scalar_tensor_tensor(
                    y_t, s_t, b1c_sb[:, 0:1], t_t, ALU.add, ALU.mult
                )
                return y_t[:, 0], y_t[:, 1]

            gconst = gconst2 = None
            if decouple or no_gates or no_pe:
                gconst = singles.tile([128, 2, 64, 2], BF16, name="gc", tag="gc")
                nc.gpsimd.memset(gconst, 1.0)
                gconst2 = singles.tile([128, 2, 64], BF16, name="gc2", tag="gc2")
                nc.gpsimd.memset(gconst2, 1.0)

            def reduces(b, psA, psB):
                """DVE reduce/hswish chain for batch b -> (yw, yh)."""
                if no_gates or no_pe:
                    if not no_pe:
                        # still drain the PSUM accumulators (cheap reduces)
                        s_t = small_pool.tile([128, 64], F32, name="s_w", tag="s_w")
                        nc.vector.reduce_sum(
                            out=s_t, in_=psA.transpose([0, 2, 1]), axis=AX.X
                        )
                        s_t2 = small_pool.tile([128, 64], F32, name="s_h", tag="s_h")
                        nc.vector.reduce_sum(out=s_t2, in_=psB, axis=AX.X)
                    return None
                return hswish2(psA, psB)

            def gates(ys):
                """PE gate matmuls + ACT sigmoids -> (gh2, gw)."""
                if ys is None:
                    return gconst, gconst2
                yw, yh = ys
                # gw gate first: the first gate multiply consumes only gw,
                # so its sigmoids must land earliest on the Act queue.
                gw_t = small_pool.tile([128, 2, 64], BF16, name="gw_t", tag="gw_t")
                gwp = ps_pool.tile([128, 2, 64], F32, name="gwp", tag="gp", bufs=2)
                for ob in range(2):
                    nc.tensor.matmul(
                        gwp[:, ob], lhsT=w3t_sb[:, ob, :], rhs=yw,
                        start=True, stop=True,
                    )
                    if sig_copy:
                        nc.scalar.activation(gw_t[:, ob, :], gwp[:, ob], AF.Copy)
                    else:
                        nc.scalar.activation(
                            gw_t[:, ob, :], gwp[:, ob], AF.Sigmoid,
                            bias=b3c_sb[:, ob : ob + 1],
                        )
                gh2_t = small_pool.tile([128, 2, 64, 2], BF16, name="gh2", tag="gh2")
                ghp = ps_pool.tile([128, 2, 64], F32, name="ghp", tag="gp", bufs=2)
                for ob in range(2):
                    nc.tensor.matmul(
                        ghp[:, ob], lhsT=w2t_sb[:, ob, :], rhs=yh,
                        start=True, stop=True,
                    )
                    for p in range(2):
                        if sig_copy:
                            nc.scalar.activation(
                                gh2_t[:, ob, :, p], ghp[:, ob], AF.Copy
                            )
                        else:
                            nc.scalar.activation(
                                gh2_t[:, ob, :, p], ghp[:, ob], AF.Sigmoid,
                                bias=b2c_sb[:, ob : ob + 1],
                            )
                if decouple:
                    return gconst, gconst2
                return gh2_t, gw_t

            def muls_store(b, gh2_t, gw_t):
                """gate multiplies (DVE 2x mode) + stores (Act ring, queued
                after this batch's sigmoids so their mul-waits block only
                later stores/sigmoids, which have a batch of slack)."""
                ov = out_d[b].rearrange("(cb c) h w -> c cb h w", cb=CB)
                # h-halved muls+stores only where the tail matters (the last
                # batch): halving costs ~0.2us of extra DVE dispatch per cb
                # but lets the final store start half a mul block earlier.
                nh = 4 if b == B_LOC - 1 else 1
                HH = H // nh
                for cb in range(CB):
                    for hh in range(nh):
                        hs = slice(hh * HH, (hh + 1) * HH)
                        xv = xtf[b][:, cb, hs]
                        if not no_mul:
                            gw_b = gw_t[:, cb, :].unsqueeze(1).broadcast_to(
                                [128, HH, W]
                            )
                            for _ in range(mul_reps):
                                nc.vector.tensor_mul(xv, xv, gw_b)
                            xp = xv.rearrange("c h (wh p) -> c h wh p", p=2)
                            g2b = gh2_t[:, cb, hs].unsqueeze(2).broadcast_to(
                                [128, HH, W // 2, 2]
                            )
                            for _ in range(mul_reps):
                                nc.vector.tensor_mul(xp, xp, g2b)
                        nc.scalar.dma_start(out=ov[:, cb, hs], in_=xv)

            # 0-depth split-gate schedule: batch b's gates are computed
            # immediately after its own reduces (gw first: its sigmoids gate
            # the first multiply), and the mul block follows directly.  The
            # DVE queue is [sm(0), M(0), sm(1), M(1), ...]; pooling (6.8us)
            # is shorter than a mul block (8.9us), so the PE stays a batch
            # ahead and the G-stalls never cascade.
            for b in range(B_LOC):
                psA, psB = pool(b)
                ys = reduces(b, psA, psB)
                gh2_t, gw_t = gates(ys)
                muls_store(b, gh2_t, gw_t)

        if unroll > 1:
            # python-unrolled repeats: sim-only stand-in for the For_i loop
            # (TimelineSim can't execute register-mode branches)
            w = load_weights()
            for _ in range(unroll):
                body(w)
        elif n_iter == 1:
            body(load_weights())
        else:
            with tc.For_i(0, n_iter, 1):
                body(load_weights())
    nc.compile()
    return nc


def get_module(n_iter: int = 1, **kwargs):
    key = (n_iter, tuple(sorted(kwargs.items())))
    if key not in _NC_CACHE:
        _NC_CACHE[key] = build_module(n_iter, **kwargs)
    return _NC_CACHE[key]


def make_in_maps(x, w1, b1, bn_gamma, bn_beta, bn_mean, bn_var, w2, b2, w3, b3):
    f64 = np.float64
    s_bn = (bn_gamma.astype(f64) / np.sqrt(bn_var.astype(f64) + 1e-5))
    w1p = (w1.astype(f64) * s_bn[:, None] / 64.0).astype(np.float32)  # [128, 256]
    b1c = ((b1.astype(f64) - bn_mean.astype(f64)) * s_bn + bn_beta.astype(f64)).astype(
        np.float32
    )
    consts = {
        "w1t": np.ascontiguousarray(
            w1p.T.reshape(CB, 128, 128).transpose(1, 0, 2)
        ).astype(NP_BF16),
        "b1c": np.ascontiguousarray(b1c.reshape(128, 1)),
        "w2t": np.ascontiguousarray(w2.T.reshape(128, 2, 128)).astype(NP_BF16),
        "b1c2": np.ascontiguousarray((b1c / 6.0 + 0.5).reshape(128, 1)),
        "b2c": np.ascontiguousarray(b2.reshape(2, 128).T),
        "w3t": np.ascontiguousarray(w3.T.reshape(128, 2, 128)).astype(NP_BF16),
        "b3c": np.ascontiguousarray(b3.reshape(2, 128).T),
    }
    xb = np.ascontiguousarray(x).astype(NP_BF16)
    in_maps = []
    for i in range(N_CORES):
        m = {"x": xb[i * B_LOC : (i + 1) * B_LOC]}
        m.update(consts)
        in_maps.append(m)
    return in_maps


def kernel(**inputs) -> np.ndarray:
    nc = get_module(1)
    in_maps = make_in_maps(**inputs)
    res = run_bass_kernel_spmd(nc, in_maps, core_ids=list(range(N_CORES)))
    out = np.concatenate([res.results[i]["out"] for i in range(N_CORES)], axis=0)
    return out.astype(np.float32)


# revision 25
# speedup vs baseline: 1.1645x; 1.1645x over previous
"""Coordinate-Attention kernel for Trainium2, data-parallel over batch on 8 NeuronCores.

Reference computation (per batch b):
  xh[c,h] = mean_w x[c,h,w]; xw[c,w] = mean_h x[c,h,w]
  y = hswish(BN(w1 @ concat(xh, xw) + b1))            # [Cm=128, 128]
  gh = sigmoid(w2 @ y[:, :64] + b2)                    # [256, 64]
  gw = sigmoid(w3 @ y[:, 64:] + b3)                    # [256, 64]
  out[c,h,w] = x[c,h,w] * gh[c,h] * gw[c,w]

Host folds BN into w1/b1 and the 1/64 pooling mean into w1. Each core
processes 4 batches; x is sharded on B across the 8 cores.

v6 ("0-depth split-gate"): wire stays bf16 (x and out cast on host; halves
HBM traffic).  A single pass is inherently serial around the DVE: the 16
gate multiplies are 2.2us each in 2x mode (HW-measured) and nothing else
can run them, so the schedule exists to keep the DVE queue dense from the
first reduce to the last mul:

  loads   all 8 x DMAs on the Sync ring (no sem waits; batch 0 split into
          half-height chunks so pooling starts ~2us earlier), while ~3.5us
          of throwaway matmuls warm the PE out of its cold 1.2 GHz pstate;
  per batch b:  pool A+B [PE] -> reduce+hswish per branch [DVE, right
          after its pass] -> gw gate then gh gate [PE matmul + ACT
          sigmoid; gw first because the first multiply consumes only gw]
          -> 2x-mode gate muls in place + stores [DVE + Act ring].

  The DVE queue is [sm(0), M(0), sm(1), M(1), ...]: pooling (7us) is
  shorter than a mul block (9us), so the PE stays a batch ahead and each
  batch's sigmoids land before the DVE reaches its muls.  Everything
  gate-chain-related runs on DVE/PE/ACT only -- gpsimd tensor ops measure
  ~2-3us each in-chain and serialized v2/v3 (80-86us vs 68us).

PSUM slots are bank-padded (8 banks): psA x3 + psB x3 + gate x2.
The gh (broadcast over w) multiply keeps the duplicated-pair gh2[c,h,2]
trick so every DVE operand's innermost AP dim is packed 2-wide (2x mode);
the last batch's muls+stores are h-quartered to shorten the final store
tail, and batch 0's loads arrive in quarter-height chunks so the first
pooling matmuls start as early as possible.
"""
import sys

for _p in ("/opt/trn_rl_repo",):
    if _p not in sys.path:
        sys.path.insert(0, _p)

import numpy as np

import concourse.bacc as bacc
import concourse.bass as bass
import concourse.tile as tile
import concourse.mybir as mybir
from concourse.bass_utils import run_bass_kernel_spmd

N_CORES = 8
B, C, H, W = 32, 256, 64, 64
B_LOC = B // N_CORES  # 4
CB = C // 128  # 2 channel blocks
F32 = mybir.dt.float32
BF16 = mybir.dt.bfloat16
NP_BF16 = mybir.dt.np(BF16)
AF = mybir.ActivationFunctionType
ALU = mybir.AluOpType
AX = mybir.AxisListType

_NC_CACHE = {}


def build_module(
    n_iter: int = 1,
    xs_bufs: int = 8,
    unroll: int = 1,
    no_mul: bool = False,    # timing-only: skip the big DVE gate multiplies
    no_gates: bool = False,  # timing-only: constant gates, skip gate compute
    no_pe: bool = False,     # timing-only: skip pooling passes
    decouple: bool = False,  # timing-only: compute gates but muls read consts
    sig_copy: bool = False,  # timing-only: ACT Copy instead of Sigmoid
    mul_reps: int = 1,       # timing-only: repeat each gate multiply N times
):
    """phase-separated wire-bf16 module. n_iter>1 wraps the workload in a
    hardware For_i loop (timing only; the graded path uses n_iter=1)."""
    nc = bacc.Bacc("TRN2", debug=False, num_devices=N_CORES)
    x_d = nc.dram_tensor("x", [B_LOC, C, H, W], BF16, kind="ExternalInput").ap()
    w1t_d = nc.dram_tensor("w1t", [128, CB, 128], BF16, kind="ExternalInput").ap()
    b1c_d = nc.dram_tensor("b1c", [128, 1], F32, kind="ExternalInput").ap()
    b1c2_d = nc.dram_tensor("b1c2", [128, 1], F32, kind="ExternalInput").ap()
    w2t_d = nc.dram_tensor("w2t", [128, 2, 128], BF16, kind="ExternalInput").ap()
    b2c_d = nc.dram_tensor("b2c", [128, 2], F32, kind="ExternalInput").ap()
    w3t_d = nc.dram_tensor("w3t", [128, 2, 128], BF16, kind="ExternalInput").ap()
    b3c_d = nc.dram_tensor("b3c", [128, 2], F32, kind="ExternalInput").ap()
    out_d = nc.dram_tensor("out", [B_LOC, C, H, W], BF16, kind="ExternalOutput").ap()

    NG = 8  # h (resp. w) rows folded per pooling matmul (512 columns)

    from contextlib import ExitStack

    with tile.TileContext(nc) as tc, ExitStack() as ctx:
        singles = ctx.enter_context(tc.tile_pool(name="singles", bufs=1))
        xs_pool = ctx.enter_context(tc.tile_pool(name="xs", bufs=xs_bufs))
        small_pool = ctx.enter_context(tc.tile_pool(name="small", bufs=4))
        ps_pool = ctx.enter_context(tc.tile_pool(name="ps", bufs=3, space="PSUM"))

        def load_weights():
            # weight DMAs on the Sync ring so the Act ring starts the
            # (critical) first-iteration x loads immediately
            w1t_sb = singles.tile([128, CB, 128], BF16, name="w1t_sb", tag="w1t_sb")
            nc.sync.dma_start(out=w1t_sb, in_=w1t_d)
            b1c_sb = singles.tile([128, 1], F32, name="b1c_sb", tag="b1c_sb")
            nc.sync.dma_start(out=b1c_sb, in_=b1c_d)
            b1c2_sb = singles.tile([128, 1], F32, name="b1c2_sb", tag="b1c2_sb")
            nc.sync.dma_start(out=b1c2_sb, in_=b1c2_d)
            w2t_sb = singles.tile([128, 2, 128], BF16, name="w2t_sb", tag="w2t_sb")
            nc.sync.dma_start(out=w2t_sb, in_=w2t_d)
            b2c_sb = singles.tile([128, 2], F32, name="b2c_sb", tag="b2c_sb")
            nc.sync.dma_start(out=b2c_sb, in_=b2c_d)
            w3t_sb = singles.tile([128, 2, 128], BF16, name="w3t_sb", tag="w3t_sb")
            nc.sync.dma_start(out=w3t_sb, in_=w3t_d)
            b3c_sb = singles.tile([128, 2], F32, name="b3c_sb", tag="b3c_sb")
            nc.sync.dma_start(out=b3c_sb, in_=b3c_d)
            return w1t_sb, b1c_sb, b1c2_sb, w2t_sb, b2c_sb, w3t_sb, b3c_sb

        def body(weights):
            w1t_sb, b1c_sb, b1c2_sb, w2t_sb, b2c_sb, w3t_sb, b3c_sb = weights

            # ---- phase L: all batch loads, Sync ring (loads alone: no sem
            # waits, so every iteration's loads dispatch the moment the SP
            # sequencer reaches them and the DMA engines always have load
            # work queued) ----
            # PE warm-up: ~3.5us of throwaway matmuls during the load ramp
            # so the PE_HAM clock gate is already at 2.4 GHz when the first
            # pooling matmul issues (cold pstate is 1.2 GHz).
            warm = singles.tile([128, 512], BF16, name="warm", tag="warm")
            nc.gpsimd.memset(warm, 0.0)
            wps = ps_pool.tile([128, 2, 64], F32, name="wps", tag="gp", bufs=2)
            for _ in range(8):
                nc.tensor.matmul(
                    wps[:, 0], lhsT=warm[:, :128], rhs=warm[:, :64],
                    start=True, stop=True,
                )

            xtf = []
            for b in range(B_LOC):
                t = xs_pool.tile([128, CB, H, W], BF16, name="xtf", tag="xtf")
                xtf.append(t)
                for cb in range(CB):
                    if b == 0:
                        # split the first batch into quarter-height chunks so
                        # pass A's first matmuls start as early as possible
                        QH = H // 4
                        for q in range(4):
                            nc.sync.dma_start(
                                out=t[:, cb, q * QH : (q + 1) * QH],
                                in_=x_d[
                                    b, cb * 128 : (cb + 1) * 128,
                                    q * QH : (q + 1) * QH,
                                ],
                            )
                    else:
                        nc.sync.dma_start(
                            out=t[:, cb], in_=x_d[b, cb * 128 : (cb + 1) * 128]
                        )

            # ---- per-batch interleaved pipeline ----
            # Emission per batch b:  pool(b) [PE] ; reduces+hswish(b) [DVE] ;
            # gate matmuls+sigmoids(b-1) [PE/ACT] ; muls+stores(b-1) [DVE/ACT].
            # DVE queue is [smalls(0), smalls(1), M(0), smalls(2), M(1), ...]
            # so the first mul block waits only on batch-0/1 pooling (not the
            # whole pooling phase), and every cross-engine link in the gate
            # chain gets a full batch period of slack.
            def pool(b):
                psA = ps_pool.tile([128, NG, W], F32, name="psA", tag="psA", bufs=3)
                psB = ps_pool.tile([128, H, NG], F32, name="psB", tag="psB", bufs=3)
                if no_pe:
                    return psA, psB
                # pass A: fold h by its low 3 bits; psA[m,i,w] accumulates
                # over cb and g with rhs a natural [c,8h,w] slice.
                for cb in range(CB):
                    for g in range(H // NG):
                        nc.tensor.matmul(
                            psA,
                            lhsT=w1t_sb[:, cb, :],
                            rhs=xtf[b][:, cb, g * NG : (g + 1) * NG, :],
                            start=(g == 0 and cb == 0),
                            stop=(g == H // NG - 1 and cb == CB - 1),
                        )
                # pass B: fold w by its low 3 bits; psB[m,h,j] accumulates
                # over g with rhs a natural [c,h,8w] slice (16B-contiguous
                # runs; a transposed-rhs view measures ~4x slower PE column
                # fetch on real HW).
                for cb in range(CB):
                    for g in range(W // NG):
                        nc.tensor.matmul(
                            psB,
                            lhsT=w1t_sb[:, cb, :],
                            rhs=xtf[b][:, cb, :, g * NG : (g + 1) * NG],
                            start=(g == 0 and cb == 0),
                            stop=(g == W // NG - 1 and cb == CB - 1),
                        )
                return psA, psB

            def hswish2(psA, psB):
                """Both branches fused: s[:,0]=reduce(psA^T) (w branch),
                s[:,1]=reduce(psB) (h branch), then ONE ts/clamp/stt pass
                over [128,2,64].  All on DVE (no cross-engine hops; the Act
                engine only ever runs Sigmoid, avoiding table reloads).
                t = clip(s/6 + (b1c/6+.5), 0, 1);  y = (s + b1c) * t."""
                s_t = small_pool.tile([128, 2, 64], F32, name="s_wh", tag="s_wh")
                nc.vector.reduce_sum(
                    out=s_t[:, 0], in_=psA.transpose([0, 2, 1]), axis=AX.X
                )
                nc.vector.reduce_sum(out=s_t[:, 1], in_=psB, axis=AX.X)
                t_t = small_pool.tile([128, 2, 64], F32, name="t_wh", tag="t_wh")
                nc.vector.tensor_scalar(
                    t_t, s_t, 1.0 / 6.0, b1c2_sb[:, 0:1], ALU.mult, ALU.add
                )
                nc.vector.tensor_scalar(t_t, t_t, 0.0, 1.0, ALU.max, ALU.min)
                y_t = small_pool.tile([128, 2, 64], BF16, name="y_wh", tag="y_wh")
                nc.vector.scalar_tensor_tensor(
                    y_t, s_t, b1c_sb[:, 0:1], t_t, ALU.add, ALU.mult
                )
                return y_t[:, 0], y_t[:, 1]

            gconst = gconst2 = None
            if decouple or no_gates or no_pe:
                gconst = singles.tile([128, 2, 64, 2], BF16, name="gc", tag="gc")
                nc.gpsimd.memset(gconst, 1.0)
                gconst2 = singles.tile([128, 2, 64], BF16, name="gc2", tag="gc2")
                nc.gpsimd.memset(gconst2, 1.0)

            def reduces(b, psA, psB):
                """DVE reduce/hswish chain for batch b -> (yw, yh)."""
                if no_gates or no_pe:
                    if not no_pe:
                        # still drain the PSUM accumulators (cheap reduces)
                        s_t = small_pool.tile([128, 64], F32, name="s_w", tag="s_w")
                        nc.vector.reduce_sum(
                            out=s_t, in_=psA.transpose([0, 2, 1]), axis=AX.X
                        )
                        s_t2 = small_pool.tile([128, 64], F32, name="s_h", tag="s_h")
                        nc.vector.reduce_sum(out=s_t2, in_=psB, axis=AX.X)
                    return None
                return hswish2(psA, psB)

            def gates(ys):
                """PE gate matmuls + ACT sigmoids -> (gh2, gw)."""
                if ys is None:
                    return gconst, gconst2
                yw, yh = ys
                # gw gate first: the first gate multiply consumes only gw,
                # so its sigmoids must land earliest on the Act queue.
                gw_t = small_pool.tile([128, 2, 64], BF16, name="gw_t", tag="gw_t")
                gwp = ps_pool.tile([128, 2, 64], F32, name="gwp", tag="gp", bufs=2)
                for ob in range(2):
                    nc.tensor.matmul(
                        gwp[:, ob], lhsT=w3t_sb[:, ob, :], rhs=yw,
                        start=True, stop=True,
                    )
                for ob in range(2):
                    if sig_copy:
                        nc.scalar.activation(gw_t[:, ob, :], gwp[:, ob], AF.Copy)
                    else:
                        nc.scalar.activation(
                            gw_t[:, ob, :], gwp[:, ob], AF.Sigmoid,
                            bias=b3c_sb[:, ob : ob + 1],
                        )
                gh2_t = small_pool.tile([128, 2, 64, 2], BF16, name="gh2", tag="gh2")
                ghp = ps_pool.tile([128, 2, 64], F32, name="ghp", tag="gp", bufs=2)
                for ob in range(2):
                    nc.tensor.matmul(
                        ghp[:, ob], lhsT=w2t_sb[:, ob, :], rhs=yh,
                        start=True, stop=True,
                    )
                for ob in range(2):
                    for p in range(2):
                        if sig_copy:
                            nc.scalar.activation(
                                gh2_t[:, ob, :, p], ghp[:, ob], AF.Copy
                            )
                        else:
                            nc.scalar.activation(
                                gh2_t[:, ob, :, p], ghp[:, ob], AF.Sigmoid,
                                bias=b2c_sb[:, ob : ob + 1],
                            )
                if decouple:
                    return gconst, gconst2
                return gh2_t, gw_t

            def muls_store(b, gh2_t, gw_t):
                """gate multiplies (DVE 2x mode) + stores (Act ring, queued
                after this batch's sigmoids so their mul-waits block only
                later stores/sigmoids, which have a batch of slack)."""
                ov = out_d[b].rearrange("(cb c) h w -> c cb h w", cb=CB)
                # h-halved muls+stores only where the tail matters (the last
                # batch): halving costs ~0.2us of extra DVE dispatch per cb
                # but lets the final store start half a mul block earlier.
                nh = 4 if b == B_LOC - 1 else 1
                HH = H // nh
                for cb in range(CB):
                    for hh in range(nh):
                        hs = slice(hh * HH, (hh + 1) * HH)
                        xv = xtf[b][:, cb, hs]
                        if not no_mul:
                            gw_b = gw_t[:, cb, :].unsqueeze(1).broadcast_to(
                                [128, HH, W]
                            )
                            for _ in range(mul_reps):
                                nc.vector.tensor_mul(xv, xv, gw_b)
                            xp = xv.rearrange("c h (wh p) -> c h wh p", p=2)
                            g2b = gh2_t[:, cb, hs].unsqueeze(2).broadcast_to(
                                [128, HH, W // 2, 2]
                            )
                            for _ in range(mul_reps):
                                nc.vector.tensor_mul(xp, xp, g2b)
                        nc.scalar.dma_start(out=ov[:, cb, hs], in_=xv)

            # 0-depth split-gate schedule: batch b's gates are computed
            # immediately after its own reduces (gw first: its sigmoids gate
            # the first multiply), and the mul block follows directly.  The
            # DVE queue is [sm(0), M(0), sm(1), M(1), ...]; pooling (6.8us)
            # is shorter than a mul block (8.9us), so the PE stays a batch
            # ahead and the G-stalls never cascade.
            for b in range(B_LOC):
                psA, psB = pool(b)
                ys = reduces(b, psA, psB)
                gh2_t, gw_t = gates(ys)
                muls_store(b, gh2_t, gw_t)

        if unroll > 1:
            # python-unrolled repeats: sim-only stand-in for the For_i loop
            # (TimelineSim can't execute register-mode branches)
            w = load_weights()
            for _ in range(unroll):
                body(w)
        elif n_iter == 1:
            body(load_weights())
        else:
            with tc.For_i(0, n_iter, 1):
                body(load_weights())
    nc.compile()
    return nc


def get_module(n_iter: int = 1, **kwargs):
    key = (n_iter, tuple(sorted(kwargs.items())))
    if key not in _NC_CACHE:
        _NC_CACHE[key] = build_module(n_iter, **kwargs)
    return _NC_CACHE[key]


def make_in_maps(x, w1, b1, bn_gamma, bn_beta, bn_mean, bn_var, w2, b2, w3, b3):
    f64 = np.float64
    s_bn = (bn_gamma.astype(f64) / np.sqrt(bn_var.astype(f64) + 1e-5))
    w1p = (w1.astype(f64) * s_bn[:, None] / 64.0).astype(np.float32)  # [128, 256]
    b1c = ((b1.astype(f64) - bn_mean.astype(f64)) * s_bn + bn_beta.astype(f64)).astype(
        np.float32
    )
    consts = {
        "w1t": np.ascontiguousarray(
            w1p.T.reshape(CB, 128, 128).transpose(1, 0, 2)
        ).astype(NP_BF16),
        "b1c": np.ascontiguousarray(b1c.reshape(128, 1)),
        "w2t": np.ascontiguousarray(w2.T.reshape(128, 2, 128)).astype(NP_BF16),
        "b1c2": np.ascontiguousarray((b1c / 6.0 + 0.5).reshape(128, 1)),
        "b2c": np.ascontiguousarray(b2.reshape(2, 128).T),
        "w3t": np.ascontiguousarray(w3.T.reshape(128, 2, 128)).astype(NP_BF16),
        "b3c": np.ascontiguousarray(b3.reshape(2, 128).T),
    }
    xb = np.ascontiguousarray(x).astype(NP_BF16)
    in_maps = []
    for i in range(N_CORES):
        m = {"x": xb[i * B_LOC : (i + 1) * B_LOC]}
        m.update(consts)
        in_maps.append(m)
    return in_maps


def kernel(**inputs) -> np.ndarray:
    nc = get_module(1)
    in_maps = make_in_maps(**inputs)
    res = run_bass_kernel_spmd(nc, in_maps, core_ids=list(range(N_CORES)))
    out = np.concatenate([res.results[i]["out"] for i in range(N_CORES)], axis=0)
    return out.astype(np.float32)


# revision 26
# speedup vs baseline: 1.1783x; 1.0118x over previous
"""Coordinate-Attention kernel for Trainium2, data-parallel over batch on 8 NeuronCores.

Reference computation (per batch b):
  xh[c,h] = mean_w x[c,h,w]; xw[c,w] = mean_h x[c,h,w]
  y = hswish(BN(w1 @ concat(xh, xw) + b1))            # [Cm=128, 128]
  gh = sigmoid(w2 @ y[:, :64] + b2)                    # [256, 64]
  gw = sigmoid(w3 @ y[:, 64:] + b3)                    # [256, 64]
  out[c,h,w] = x[c,h,w] * gh[c,h] * gw[c,w]

Host folds BN into w1/b1 and the 1/64 pooling mean into w1. Each core
processes 4 batches; x is sharded on B across the 8 cores.

v6 ("0-depth split-gate"): wire stays bf16 (x and out cast on host; halves
HBM traffic).  A single pass is inherently serial around the DVE: the 16
gate multiplies are 2.2us each in 2x mode (HW-measured) and nothing else
can run them, so the schedule exists to keep the DVE queue dense from the
first reduce to the last mul:

  loads   all 8 x DMAs on the Sync ring (no sem waits; batch 0 split into
          half-height chunks so pooling starts ~2us earlier), while ~3.5us
          of throwaway matmuls warm the PE out of its cold 1.2 GHz pstate;
  per batch b:  pool A+B [PE] -> reduce+hswish per branch [DVE, right
          after its pass] -> gw gate then gh gate [PE matmul + ACT
          sigmoid; gw first because the first multiply consumes only gw]
          -> 2x-mode gate muls in place + stores [DVE + Act ring].

  The DVE queue is [sm(0), M(0), sm(1), M(1), ...]: pooling (7us) is
  shorter than a mul block (9us), so the PE stays a batch ahead and each
  batch's sigmoids land before the DVE reaches its muls.  Everything
  gate-chain-related runs on DVE/PE/ACT only -- gpsimd tensor ops measure
  ~2-3us each in-chain and serialized v2/v3 (80-86us vs 68us).

PSUM slots are bank-padded (8 banks): psA x3 + psB x3 + gate x2.
The gh (broadcast over w) multiply keeps the duplicated-pair gh2[c,h,2]
trick so every DVE operand's innermost AP dim is packed 2-wide (2x mode);
the last batch's muls+stores are h-quartered to shorten the final store
tail, and batch 0's loads arrive in quarter-height chunks so the first
pooling matmuls start as early as possible.
"""
import sys

for _p in ("/opt/trn_rl_repo",):
    if _p not in sys.path:
        sys.path.insert(0, _p)

import numpy as np

import concourse.bacc as bacc
import concourse.bass as bass
import concourse.tile as tile
import concourse.mybir as mybir
from concourse.bass_utils import run_bass_kernel_spmd

N_CORES = 8
B, C, H, W = 32, 256, 64, 64
B_LOC = B // N_CORES  # 4
CB = C // 128  # 2 channel blocks
F32 = mybir.dt.float32
BF16 = mybir.dt.bfloat16
NP_BF16 = mybir.dt.np(BF16)
AF = mybir.ActivationFunctionType
ALU = mybir.AluOpType
AX = mybir.AxisListType

_NC_CACHE = {}


def build_module(
    n_iter: int = 1,
    xs_bufs: int = 8,
    unroll: int = 1,
    no_mul: bool = False,    # timing-only: skip the big DVE gate multiplies
    no_gates: bool = False,  # timing-only: constant gates, skip gate compute
    no_pe: bool = False,     # timing-only: skip pooling passes
    decouple: bool = False,  # timing-only: compute gates but muls read consts
    sig_copy: bool = False,  # timing-only: ACT Copy instead of Sigmoid
    mul_reps: int = 1,       # timing-only: repeat each gate multiply N times
    b0_split: bool = True,   # split batch-0 cb0 load for an earlier pool start
):
    """phase-separated wire-bf16 module. n_iter>1 wraps the workload in a
    hardware For_i loop (timing only; the graded path uses n_iter=1)."""
    nc = bacc.Bacc("TRN2", debug=False, num_devices=N_CORES)
    x_d = nc.dram_tensor("x", [B_LOC, C, H, W], BF16, kind="ExternalInput").ap()
    w1t_d = nc.dram_tensor("w1t", [128, CB, 128], BF16, kind="ExternalInput").ap()
    b1c_d = nc.dram_tensor("b1c", [128, 1], F32, kind="ExternalInput").ap()
    b1c2_d = nc.dram_tensor("b1c2", [128, 1], F32, kind="ExternalInput").ap()
    w2t_d = nc.dram_tensor("w2t", [128, 2, 128], BF16, kind="ExternalInput").ap()
    b2c_d = nc.dram_tensor("b2c", [128, 2], F32, kind="ExternalInput").ap()
    w3t_d = nc.dram_tensor("w3t", [128, 2, 128], BF16, kind="ExternalInput").ap()
    b3c_d = nc.dram_tensor("b3c", [128, 2], F32, kind="ExternalInput").ap()
    out_d = nc.dram_tensor("out", [B_LOC, C, H, W], BF16, kind="ExternalOutput").ap()

    NG = 8  # h (resp. w) rows folded per pooling matmul (512 columns)

    from contextlib import ExitStack

    with tile.TileContext(nc) as tc, ExitStack() as ctx:
        singles = ctx.enter_context(tc.tile_pool(name="singles", bufs=1))
        xs_pool = ctx.enter_context(tc.tile_pool(name="xs", bufs=xs_bufs))
        small_pool = ctx.enter_context(tc.tile_pool(name="small", bufs=4))
        ps_pool = ctx.enter_context(tc.tile_pool(name="ps", bufs=3, space="PSUM"))

        def load_weights():
            # weight DMAs on the Sync ring so the Act ring starts the
            # (critical) first-iteration x loads immediately
            w1t_sb = singles.tile([128, CB, 128], BF16, name="w1t_sb", tag="w1t_sb")
            nc.sync.dma_start(out=w1t_sb, in_=w1t_d)
            b1c_sb = singles.tile([128, 1], F32, name="b1c_sb", tag="b1c_sb")
            nc.sync.dma_start(out=b1c_sb, in_=b1c_d)
            b1c2_sb = singles.tile([128, 1], F32, name="b1c2_sb", tag="b1c2_sb")
            nc.sync.dma_start(out=b1c2_sb, in_=b1c2_d)
            w2t_sb = singles.tile([128, 2, 128], BF16, name="w2t_sb", tag="w2t_sb")
            nc.sync.dma_start(out=w2t_sb, in_=w2t_d)
            b2c_sb = singles.tile([128, 2], F32, name="b2c_sb", tag="b2c_sb")
            nc.sync.dma_start(out=b2c_sb, in_=b2c_d)
            w3t_sb = singles.tile([128, 2, 128], BF16, name="w3t_sb", tag="w3t_sb")
            nc.sync.dma_start(out=w3t_sb, in_=w3t_d)
            b3c_sb = singles.tile([128, 2], F32, name="b3c_sb", tag="b3c_sb")
            nc.sync.dma_start(out=b3c_sb, in_=b3c_d)
            return w1t_sb, b1c_sb, b1c2_sb, w2t_sb, b2c_sb, w3t_sb, b3c_sb

        def body(weights):
            w1t_sb, b1c_sb, b1c2_sb, w2t_sb, b2c_sb, w3t_sb, b3c_sb = weights

            # ---- phase L: all batch loads, Sync ring (loads alone: no sem
            # waits, so every iteration's loads dispatch the moment the SP
            # sequencer reaches them and the DMA engines always have load
            # work queued) ----
            # PE warm-up: ~3.5us of throwaway matmuls during the load ramp
            # so the PE_HAM clock gate is already at 2.4 GHz when the first
            # pooling matmul issues (cold pstate is 1.2 GHz).
            warm = singles.tile([128, 512], BF16, name="warm", tag="warm")
            nc.gpsimd.memset(warm, 0.0)
            wps = ps_pool.tile([128, 2, 64], F32, name="wps", tag="gp", bufs=2)
            for _ in range(8):
                nc.tensor.matmul(
                    wps[:, 0], lhsT=warm[:, :128], rhs=warm[:, :64],
                    start=True, stop=True,
                )

            xtf = []
            for b in range(B_LOC):
                t = xs_pool.tile([128, CB, H, W], BF16, name="xtf", tag="xtf")
                xtf.append(t)
                for cb in range(CB):
                    if b == 0 and cb == 0 and b0_split:
                        # cb0 in half-height chunks so pass A starts early;
                        # cb1 stays one 1MB transfer (sub-1MB DMAs lose
                        # descriptor efficiency, and pass B is gated on the
                        # full tile, so the last-arriving chunk should be a
                        # full-rate transfer)
                        for q in range(2):
                            nc.sync.dma_start(
                                out=t[:, cb, q * 32 : (q + 1) * 32],
                                in_=x_d[
                                    b, cb * 128 : (cb + 1) * 128,
                                    q * 32 : (q + 1) * 32,
                                ],
                            )
                    else:
                        nc.sync.dma_start(
                            out=t[:, cb], in_=x_d[b, cb * 128 : (cb + 1) * 128]
                        )

            # ---- per-batch interleaved pipeline ----
            # Emission per batch b:  pool(b) [PE] ; reduces+hswish(b) [DVE] ;
            # gate matmuls+sigmoids(b-1) [PE/ACT] ; muls+stores(b-1) [DVE/ACT].
            # DVE queue is [smalls(0), smalls(1), M(0), smalls(2), M(1), ...]
            # so the first mul block waits only on batch-0/1 pooling (not the
            # whole pooling phase), and every cross-engine link in the gate
            # chain gets a full batch period of slack.
            def pool(b):
                psA = ps_pool.tile([128, NG, W], F32, name="psA", tag="psA", bufs=3)
                psB = ps_pool.tile([128, H, NG], F32, name="psB", tag="psB", bufs=3)
                if no_pe:
                    return psA, psB
                # pass A: fold h by its low 3 bits; psA[m,i,w] accumulates
                # over cb and g with rhs a natural [c,8h,w] slice.
                for cb in range(CB):
                    for g in range(H // NG):
                        nc.tensor.matmul(
                            psA,
                            lhsT=w1t_sb[:, cb, :],
                            rhs=xtf[b][:, cb, g * NG : (g + 1) * NG, :],
                            start=(g == 0 and cb == 0),
                            stop=(g == H // NG - 1 and cb == CB - 1),
                        )
                # pass B: fold w by its low 3 bits; psB[m,h,j] accumulates
                # over g with rhs a natural [c,h,8w] slice (16B-contiguous
                # runs; a transposed-rhs view measures ~4x slower PE column
                # fetch on real HW).
                for cb in range(CB):
                    for g in range(W // NG):
                        nc.tensor.matmul(
                            psB,
                            lhsT=w1t_sb[:, cb, :],
                            rhs=xtf[b][:, cb, :, g * NG : (g + 1) * NG],
                            start=(g == 0 and cb == 0),
                            stop=(g == W // NG - 1 and cb == CB - 1),
                        )
                return psA, psB

            def hswish2(psA, psB):
                """Both branches fused: s[:,0]=reduce(psA^T) (w branch),
                s[:,1]=reduce(psB) (h branch), then ONE ts/clamp/stt pass
                over [128,2,64].  All on DVE (no cross-engine hops; the Act
                engine only ever runs Sigmoid, avoiding table reloads).
                t = clip(s/6 + (b1c/6+.5), 0, 1);  y = (s + b1c) * t."""
                s_t = small_pool.tile([128, 2, 64], F32, name="s_wh", tag="s_wh")
                nc.vector.reduce_sum(
                    out=s_t[:, 0], in_=psA.transpose([0, 2, 1]), axis=AX.X
                )
                nc.vector.reduce_sum(out=s_t[:, 1], in_=psB, axis=AX.X)
                t_t = small_pool.tile([128, 2, 64], F32, name="t_wh", tag="t_wh")
                nc.vector.tensor_scalar(
                    t_t, s_t, 1.0 / 6.0, b1c2_sb[:, 0:1], ALU.mult, ALU.add
                )
                nc.vector.tensor_scalar(t_t, t_t, 0.0, 1.0, ALU.max, ALU.min)
                y_t = small_pool.tile([128, 2, 64], BF16, name="y_wh", tag="y_wh")
                nc.vector.scalar_tensor_tensor(
                    y_t, s_t, b1c_sb[:, 0:1], t_t, ALU.add, ALU.mult
                )
                return y_t[:, 0], y_t[:, 1]

            gconst = gconst2 = None
            if decouple or no_gates or no_pe:
                gconst = singles.tile([128, 2, 64, 2], BF16, name="gc", tag="gc")
                nc.gpsimd.memset(gconst, 1.0)
                gconst2 = singles.tile([128, 2, 64], BF16, name="gc2", tag="gc2")
                nc.gpsimd.memset(gconst2, 1.0)

            def reduces(b, psA, psB):
                """DVE reduce/hswish chain for batch b -> (yw, yh)."""
                if no_gates or no_pe:
                    if not no_pe:
                        # still drain the PSUM accumulators (cheap reduces)
                        s_t = small_pool.tile([128, 64], F32, name="s_w", tag="s_w")
                        nc.vector.reduce_sum(
                            out=s_t, in_=psA.transpose([0, 2, 1]), axis=AX.X
                        )
                        s_t2 = small_pool.tile([128, 64], F32, name="s_h", tag="s_h")
                        nc.vector.reduce_sum(out=s_t2, in_=psB, axis=AX.X)
                    return None
                return hswish2(psA, psB)

            def gates(ys):
                """PE gate matmuls + ACT sigmoids -> (gh2, gw)."""
                if ys is None:
                    return gconst, gconst2
                yw, yh = ys
                # gw gate first: the first gate multiply consumes only gw,
                # so its sigmoids must land earliest on the Act queue.
                gw_t = small_pool.tile([128, 2, 64], BF16, name="gw_t", tag="gw_t")
                gwp = ps_pool.tile([128, 2, 64], F32, name="gwp", tag="gp", bufs=2)
                for ob in range(2):
                    nc.tensor.matmul(
                        gwp[:, ob], lhsT=w3t_sb[:, ob, :], rhs=yw,
                        start=True, stop=True,
                    )
                for ob in range(2):
                    if sig_copy:
                        nc.scalar.activation(gw_t[:, ob, :], gwp[:, ob], AF.Copy)
                    else:
                        nc.scalar.activation(
                            gw_t[:, ob, :], gwp[:, ob], AF.Sigmoid,
                            bias=b3c_sb[:, ob : ob + 1],
                        )
                gh2_t = small_pool.tile([128, 2, 64, 2], BF16, name="gh2", tag="gh2")
                ghp = ps_pool.tile([128, 2, 64], F32, name="ghp", tag="gp", bufs=2)
                for ob in range(2):
                    nc.tensor.matmul(
                        ghp[:, ob], lhsT=w2t_sb[:, ob, :], rhs=yh,
                        start=True, stop=True,
                    )
                for ob in range(2):
                    for p in range(2):
                        if sig_copy:
                            nc.scalar.activation(
                                gh2_t[:, ob, :, p], ghp[:, ob], AF.Copy
                            )
                        else:
                            nc.scalar.activation(
                                gh2_t[:, ob, :, p], ghp[:, ob], AF.Sigmoid,
                                bias=b2c_sb[:, ob : ob + 1],
                            )
                if decouple:
                    return gconst, gconst2
                return gh2_t, gw_t

            def muls_store(b, gh2_t, gw_t):
                """gate multiplies (DVE 2x mode) + stores (Act ring, queued
                after this batch's sigmoids so their mul-waits block only
                later stores/sigmoids, which have a batch of slack)."""
                ov = out_d[b].rearrange("(cb c) h w -> c cb h w", cb=CB)
                # h-halved muls+stores only where the tail matters (the last
                # batch): halving costs ~0.2us of extra DVE dispatch per cb
                # but lets the final store start half a mul block earlier.
                nh = 4 if b == B_LOC - 1 else 1
                HH = H // nh
                for cb in range(CB):
                    for hh in range(nh):
                        hs = slice(hh * HH, (hh + 1) * HH)
                        xv = xtf[b][:, cb, hs]
                        if not no_mul:
                            gw_b = gw_t[:, cb, :].unsqueeze(1).broadcast_to(
                                [128, HH, W]
                            )
                            for _ in range(mul_reps):
                                nc.vector.tensor_mul(xv, xv, gw_b)
                            xp = xv.rearrange("c h (wh p) -> c h wh p", p=2)
                            g2b = gh2_t[:, cb, hs].unsqueeze(2).broadcast_to(
                                [128, HH, W // 2, 2]
                            )
                            for _ in range(mul_reps):
                                nc.vector.tensor_mul(xp, xp, g2b)
                        nc.scalar.dma_start(out=ov[:, cb, hs], in_=xv)

            # 0-depth split-gate schedule: batch b's gates are computed
            # immediately after its own reduces (gw first: its sigmoids gate
            # the first multiply), and the mul block follows directly.  The
            # DVE queue is [sm(0), M(0), sm(1), M(1), ...]; pooling (6.8us)
            # is shorter than a mul block (8.9us), so the PE stays a batch
            # ahead and the G-stalls never cascade.
            for b in range(B_LOC):
                psA, psB = pool(b)
                ys = reduces(b, psA, psB)
                gh2_t, gw_t = gates(ys)
                muls_store(b, gh2_t, gw_t)

        if unroll > 1:
            # python-unrolled repeats: sim-only stand-in for the For_i loop
            # (TimelineSim can't execute register-mode branches)
            w = load_weights()
            for _ in range(unroll):
                body(w)
        elif n_iter == 1:
            body(load_weights())
        else:
            with tc.For_i(0, n_iter, 1):
                body(load_weights())
    nc.compile()
    return nc


def get_module(n_iter: int = 1, **kwargs):
    key = (n_iter, tuple(sorted(kwargs.items())))
    if key not in _NC_CACHE:
        _NC_CACHE[key] = build_module(n_iter, **kwargs)
    return _NC_CACHE[key]


def make_in_maps(x, w1, b1, bn_gamma, bn_beta, bn_mean, bn_var, w2, b2, w3, b3):
    f64 = np.float64
    s_bn = (bn_gamma.astype(f64) / np.sqrt(bn_var.astype(f64) + 1e-5))
    w1p = (w1.astype(f64) * s_bn[:, None] / 64.0).astype(np.float32)  # [128, 256]
    b1c = ((b1.astype(f64) - bn_mean.astype(f64)) * s_bn + bn_beta.astype(f64)).astype(
        np.float32
    )
    consts = {
        "w1t": np.ascontiguousarray(
            w1p.T.reshape(CB, 128, 128).transpose(1, 0, 2)
        ).astype(NP_BF16),
        "b1c": np.ascontiguousarray(b1c.reshape(128, 1)),
        "w2t": np.ascontiguousarray(w2.T.reshape(128, 2, 128)).astype(NP_BF16),
        "b1c2": np.ascontiguousarray((b1c / 6.0 + 0.5).reshape(128, 1)),
        "b2c": np.ascontiguousarray(b2.reshape(2, 128).T),
        "w3t": np.ascontiguousarray(w3.T.reshape(128, 2, 128)).astype(NP_BF16),
        "b3c": np.ascontiguousarray(b3.reshape(2, 128).T),
    }
    xb = np.ascontiguousarray(x).astype(NP_BF16)
    in_maps = []
    for i in range(N_CORES):
        m = {"x": xb[i * B_LOC : (i + 1) * B_LOC]}
        m.update(consts)
        in_maps.append(m)
    return in_maps


def kernel(**inputs) -> np.ndarray:
    nc = get_module(1)
    in_maps = make_in_maps(**inputs)
    res = run_bass_kernel_spmd(nc, in_maps, core_ids=list(range(N_CORES)))
    out = np.concatenate([res.results[i]["out"] for i in range(N_CORES)], axis=0)
    return out.astype(np.float32)


# revision 27
# speedup vs baseline: 1.2015x; 1.0197x over previous
"""Coordinate-Attention kernel for Trainium2, data-parallel over batch on 8 NeuronCores.

Reference computation (per batch b):
  xh[c,h] = mean_w x[c,h,w]; xw[c,w] = mean_h x[c,h,w]
  y = hswish(BN(w1 @ concat(xh, xw) + b1))            # [Cm=128, 128]
  gh = sigmoid(w2 @ y[:, :64] + b2)                    # [256, 64]
  gw = sigmoid(w3 @ y[:, 64:] + b3)                    # [256, 64]
  out[c,h,w] = x[c,h,w] * gh[c,h] * gw[c,w]

Host folds BN into w1/b1 and the 1/64 pooling mean into w1. Each core
processes 4 batches; x is sharded on B across the 8 cores.

v6 ("0-depth split-gate"): wire stays bf16 (x and out cast on host; halves
HBM traffic).  A single pass is inherently serial around the DVE: the 16
gate multiplies are 2.2us each in 2x mode (HW-measured) and nothing else
can run them, so the schedule exists to keep the DVE queue dense from the
first reduce to the last mul:

  loads   all 8 x DMAs on the Sync ring (no sem waits; batch 0 split into
          half-height chunks so pooling starts ~2us earlier), while ~3.5us
          of throwaway matmuls warm the PE out of its cold 1.2 GHz pstate;
  per batch b:  pool A+B [PE] -> reduce+hswish per branch [DVE, right
          after its pass] -> gw gate then gh gate [PE matmul + ACT
          sigmoid; gw first because the first multiply consumes only gw]
          -> 2x-mode gate muls in place + stores [DVE + Act ring].

  The DVE queue is [sm(0), M(0), sm(1), M(1), ...]: pooling (7us) is
  shorter than a mul block (9us), so the PE stays a batch ahead and each
  batch's sigmoids land before the DVE reaches its muls.  Everything
  gate-chain-related runs on DVE/PE/ACT only -- gpsimd tensor ops measure
  ~2-3us each in-chain and serialized v2/v3 (80-86us vs 68us).

PSUM slots are bank-padded (8 banks): psA x3 + psB x3 + gate x2.
The gh (broadcast over w) multiply keeps the duplicated-pair gh2[c,h,2]
trick so every DVE operand's innermost AP dim is packed 2-wide (2x mode);
the last batch's muls+stores are h-quartered to shorten the final store
tail, and batch 0's loads arrive in quarter-height chunks so the first
pooling matmuls start as early as possible.
"""
import sys

for _p in ("/opt/trn_rl_repo",):
    if _p not in sys.path:
        sys.path.insert(0, _p)

import numpy as np

import concourse.bacc as bacc
import concourse.bass as bass
import concourse.tile as tile
import concourse.mybir as mybir
from concourse.bass_utils import run_bass_kernel_spmd

N_CORES = 8
B, C, H, W = 32, 256, 64, 64
B_LOC = B // N_CORES  # 4
CB = C // 128  # 2 channel blocks
F32 = mybir.dt.float32
BF16 = mybir.dt.bfloat16
NP_BF16 = mybir.dt.np(BF16)
AF = mybir.ActivationFunctionType
ALU = mybir.AluOpType
AX = mybir.AxisListType

_NC_CACHE = {}


def build_module(
    n_iter: int = 1,
    xs_bufs: int = 8,
    unroll: int = 1,
    no_mul: bool = False,    # timing-only: skip the big DVE gate multiplies
    no_gates: bool = False,  # timing-only: constant gates, skip gate compute
    no_pe: bool = False,     # timing-only: skip pooling passes
    decouple: bool = False,  # timing-only: compute gates but muls read consts
    sig_copy: bool = False,  # timing-only: ACT Copy instead of Sigmoid
    mul_reps: int = 1,       # timing-only: repeat each gate multiply N times
    b0_split: bool = True,   # split batch-0 cb0 load for an earlier pool start
    hsw_fused: bool = True,  # fuse both branches' hswish into wide DVE ops
):
    """phase-separated wire-bf16 module. n_iter>1 wraps the workload in a
    hardware For_i loop (timing only; the graded path uses n_iter=1)."""
    nc = bacc.Bacc("TRN2", debug=False, num_devices=N_CORES)
    x_d = nc.dram_tensor("x", [B_LOC, C, H, W], BF16, kind="ExternalInput").ap()
    w1t_d = nc.dram_tensor("w1t", [128, CB, 128], BF16, kind="ExternalInput").ap()
    b1c_d = nc.dram_tensor("b1c", [128, 1], F32, kind="ExternalInput").ap()
    b1c2_d = nc.dram_tensor("b1c2", [128, 1], F32, kind="ExternalInput").ap()
    w2t_d = nc.dram_tensor("w2t", [128, 2, 128], BF16, kind="ExternalInput").ap()
    b2c_d = nc.dram_tensor("b2c", [128, 2], F32, kind="ExternalInput").ap()
    w3t_d = nc.dram_tensor("w3t", [128, 2, 128], BF16, kind="ExternalInput").ap()
    b3c_d = nc.dram_tensor("b3c", [128, 2], F32, kind="ExternalInput").ap()
    out_d = nc.dram_tensor("out", [B_LOC, C, H, W], BF16, kind="ExternalOutput").ap()

    NG = 8  # h (resp. w) rows folded per pooling matmul (512 columns)

    from contextlib import ExitStack

    with tile.TileContext(nc) as tc, ExitStack() as ctx:
        singles = ctx.enter_context(tc.tile_pool(name="singles", bufs=1))
        xs_pool = ctx.enter_context(tc.tile_pool(name="xs", bufs=xs_bufs))
        small_pool = ctx.enter_context(tc.tile_pool(name="small", bufs=4))
        ps_pool = ctx.enter_context(tc.tile_pool(name="ps", bufs=3, space="PSUM"))

        def load_weights():
            # weight DMAs on the Sync ring so the Act ring starts the
            # (critical) first-iteration x loads immediately
            w1t_sb = singles.tile([128, CB, 128], BF16, name="w1t_sb", tag="w1t_sb")
            nc.sync.dma_start(out=w1t_sb, in_=w1t_d)
            b1c_sb = singles.tile([128, 1], F32, name="b1c_sb", tag="b1c_sb")
            nc.sync.dma_start(out=b1c_sb, in_=b1c_d)
            b1c2_sb = singles.tile([128, 1], F32, name="b1c2_sb", tag="b1c2_sb")
            nc.sync.dma_start(out=b1c2_sb, in_=b1c2_d)
            w2t_sb = singles.tile([128, 2, 128], BF16, name="w2t_sb", tag="w2t_sb")
            nc.sync.dma_start(out=w2t_sb, in_=w2t_d)
            b2c_sb = singles.tile([128, 2], F32, name="b2c_sb", tag="b2c_sb")
            nc.sync.dma_start(out=b2c_sb, in_=b2c_d)
            w3t_sb = singles.tile([128, 2, 128], BF16, name="w3t_sb", tag="w3t_sb")
            nc.sync.dma_start(out=w3t_sb, in_=w3t_d)
            b3c_sb = singles.tile([128, 2], F32, name="b3c_sb", tag="b3c_sb")
            nc.sync.dma_start(out=b3c_sb, in_=b3c_d)
            return w1t_sb, b1c_sb, b1c2_sb, w2t_sb, b2c_sb, w3t_sb, b3c_sb

        def body(weights):
            w1t_sb, b1c_sb, b1c2_sb, w2t_sb, b2c_sb, w3t_sb, b3c_sb = weights

            # ---- phase L: all batch loads, Sync ring (loads alone: no sem
            # waits, so every iteration's loads dispatch the moment the SP
            # sequencer reaches them and the DMA engines always have load
            # work queued) ----
            # PE warm-up: ~3.5us of throwaway matmuls during the load ramp
            # so the PE_HAM clock gate is already at 2.4 GHz when the first
            # pooling matmul issues (cold pstate is 1.2 GHz).
            warm = singles.tile([128, 512], BF16, name="warm", tag="warm")
            nc.gpsimd.memset(warm, 0.0)
            wps = ps_pool.tile([128, 2, 64], F32, name="wps", tag="gp", bufs=2)
            for _ in range(8):
                nc.tensor.matmul(
                    wps[:, 0], lhsT=warm[:, :128], rhs=warm[:, :64],
                    start=True, stop=True,
                )

            xtf = []
            for b in range(B_LOC):
                t = xs_pool.tile([128, CB, H, W], BF16, name="xtf", tag="xtf")
                xtf.append(t)
                for cb in range(CB):
                    if b == 0 and cb == 0 and b0_split:
                        # cb0 in half-height chunks so pass A starts early;
                        # cb1 stays one 1MB transfer (sub-1MB DMAs lose
                        # descriptor efficiency, and pass B is gated on the
                        # full tile, so the last-arriving chunk should be a
                        # full-rate transfer)
                        for q in range(2):
                            nc.sync.dma_start(
                                out=t[:, cb, q * 32 : (q + 1) * 32],
                                in_=x_d[
                                    b, cb * 128 : (cb + 1) * 128,
                                    q * 32 : (q + 1) * 32,
                                ],
                            )
                    else:
                        nc.sync.dma_start(
                            out=t[:, cb], in_=x_d[b, cb * 128 : (cb + 1) * 128]
                        )

            # ---- per-batch interleaved pipeline ----
            # Emission per batch b:  pool(b) [PE] ; reduces+hswish(b) [DVE] ;
            # gate matmuls+sigmoids(b-1) [PE/ACT] ; muls+stores(b-1) [DVE/ACT].
            # DVE queue is [smalls(0), smalls(1), M(0), smalls(2), M(1), ...]
            # so the first mul block waits only on batch-0/1 pooling (not the
            # whole pooling phase), and every cross-engine link in the gate
            # chain gets a full batch period of slack.
            def pool(b):
                psA = ps_pool.tile([128, NG, W], F32, name="psA", tag="psA", bufs=3)
                psB = ps_pool.tile([128, H, NG], F32, name="psB", tag="psB", bufs=3)
                if no_pe:
                    return psA, psB
                # pass A: fold h by its low 3 bits; psA[m,i,w] accumulates
                # over cb and g with rhs a natural [c,8h,w] slice.
                for cb in range(CB):
                    for g in range(H // NG):
                        nc.tensor.matmul(
                            psA,
                            lhsT=w1t_sb[:, cb, :],
                            rhs=xtf[b][:, cb, g * NG : (g + 1) * NG, :],
                            start=(g == 0 and cb == 0),
                            stop=(g == H // NG - 1 and cb == CB - 1),
                        )
                # pass B: fold w by its low 3 bits; psB[m,h,j] accumulates
                # over g with rhs a natural [c,h,8w] slice (16B-contiguous
                # runs; a transposed-rhs view measures ~4x slower PE column
                # fetch on real HW).
                for cb in range(CB):
                    for g in range(W // NG):
                        nc.tensor.matmul(
                            psB,
                            lhsT=w1t_sb[:, cb, :],
                            rhs=xtf[b][:, cb, :, g * NG : (g + 1) * NG],
                            start=(g == 0 and cb == 0),
                            stop=(g == W // NG - 1 and cb == CB - 1),
                        )
                return psA, psB

            def hswish2(psA, psB):
                """Both branches fused: s[:,0]=reduce(psA^T) (w branch),
                s[:,1]=reduce(psB) (h branch), then ONE ts/clamp/stt pass
                over [128,2,64].  All on DVE (no cross-engine hops; the Act
                engine only ever runs Sigmoid, avoiding table reloads).
                t = clip(s/6 + (b1c/6+.5), 0, 1);  y = (s + b1c) * t."""
                s_t = small_pool.tile([128, 2, 64], F32, name="s_wh", tag="s_wh")
                nc.vector.reduce_sum(
                    out=s_t[:, 0], in_=psA.transpose([0, 2, 1]), axis=AX.X
                )
                nc.vector.reduce_sum(out=s_t[:, 1], in_=psB, axis=AX.X)
                t_t = small_pool.tile([128, 2, 64], F32, name="t_wh", tag="t_wh")
                nc.vector.tensor_scalar(
                    t_t, s_t, 1.0 / 6.0, b1c2_sb[:, 0:1], ALU.mult, ALU.add
                )
                nc.vector.tensor_scalar(t_t, t_t, 0.0, 1.0, ALU.max, ALU.min)
                y_t = small_pool.tile([128, 2, 64], BF16, name="y_wh", tag="y_wh")
                nc.vector.scalar_tensor_tensor(
                    y_t, s_t, b1c_sb[:, 0:1], t_t, ALU.add, ALU.mult
                )
                return y_t[:, 0], y_t[:, 1]

            def hswish1(z_ps, tagp, transpose_ps):
                """Single-branch chain (w branch first at the call site): yw
                comes out ~1.3us before the h-branch ops finish, giving the
                gwmm->sigmoid chain a head start on the first multiply."""
                s_t = small_pool.tile([128, 64], F32, name=f"s_{tagp}", tag=f"s_{tagp}")
                nc.vector.reduce_sum(
                    out=s_t,
                    in_=z_ps.transpose([0, 2, 1]) if transpose_ps else z_ps,
                    axis=AX.X,
                )
                t_t = small_pool.tile([128, 64], F32, name=f"t_{tagp}", tag=f"t_{tagp}")
                nc.vector.tensor_scalar(
                    t_t, s_t, 1.0 / 6.0, b1c2_sb[:, 0:1], ALU.mult, ALU.add
                )
                nc.vector.tensor_scalar(t_t, t_t, 0.0, 1.0, ALU.max, ALU.min)
                y_t = small_pool.tile([128, 64], BF16, name=f"y_{tagp}", tag=f"y_{tagp}")
                nc.vector.scalar_tensor_tensor(
                    y_t, s_t, b1c_sb[:, 0:1], t_t, ALU.add, ALU.mult
                )
                return y_t

            gconst = gconst2 = None
            if decouple or no_gates or no_pe:
                gconst = singles.tile([128, 2, 64, 2], BF16, name="gc", tag="gc")
                nc.gpsimd.memset(gconst, 1.0)
                gconst2 = singles.tile([128, 2, 64], BF16, name="gc2", tag="gc2")
                nc.gpsimd.memset(gconst2, 1.0)

            def reduces(b, psA, psB):
                """DVE reduce/hswish chain for batch b -> (yw, yh)."""
                if no_gates or no_pe:
                    if not no_pe:
                        # still drain the PSUM accumulators (cheap reduces)
                        s_t = small_pool.tile([128, 64], F32, name="s_w", tag="s_w")
                        nc.vector.reduce_sum(
                            out=s_t, in_=psA.transpose([0, 2, 1]), axis=AX.X
                        )
                        s_t2 = small_pool.tile([128, 64], F32, name="s_h", tag="s_h")
                        nc.vector.reduce_sum(out=s_t2, in_=psB, axis=AX.X)
                    return None
                if hsw_fused:
                    return hswish2(psA, psB)
                yw = hswish1(psA, "w", transpose_ps=True)
                yh = hswish1(psB, "h", transpose_ps=False)
                return yw, yh

            def gates(ys):
                """PE gate matmuls + ACT sigmoids -> (gh2, gw)."""
                if ys is None:
                    return gconst, gconst2
                yw, yh = ys
                # gw gate first: the first gate multiply consumes only gw,
                # so its sigmoids must land earliest on the Act queue.
                gw_t = small_pool.tile([128, 2, 64], BF16, name="gw_t", tag="gw_t")
                gwp = ps_pool.tile([128, 2, 64], F32, name="gwp", tag="gp", bufs=2)
                for ob in range(2):
                    nc.tensor.matmul(
                        gwp[:, ob], lhsT=w3t_sb[:, ob, :], rhs=yw,
                        start=True, stop=True,
                    )
                for ob in range(2):
                    if sig_copy:
                        nc.scalar.activation(gw_t[:, ob, :], gwp[:, ob], AF.Copy)
                    else:
                        nc.scalar.activation(
                            gw_t[:, ob, :], gwp[:, ob], AF.Sigmoid,
                            bias=b3c_sb[:, ob : ob + 1],
                        )
                gh2_t = small_pool.tile([128, 2, 64, 2], BF16, name="gh2", tag="gh2")
                ghp = ps_pool.tile([128, 2, 64], F32, name="ghp", tag="gp", bufs=2)
                for ob in range(2):
                    nc.tensor.matmul(
                        ghp[:, ob], lhsT=w2t_sb[:, ob, :], rhs=yh,
                        start=True, stop=True,
                    )
                for ob in range(2):
                    for p in range(2):
                        if sig_copy:
                            nc.scalar.activation(
                                gh2_t[:, ob, :, p], ghp[:, ob], AF.Copy
                            )
                        else:
                            nc.scalar.activation(
                                gh2_t[:, ob, :, p], ghp[:, ob], AF.Sigmoid,
                                bias=b2c_sb[:, ob : ob + 1],
                            )
                if decouple:
                    return gconst, gconst2
                return gh2_t, gw_t

            def muls_store(b, gh2_t, gw_t):
                """gate multiplies (DVE 2x mode) + stores (Act ring, queued
                after this batch's sigmoids so their mul-waits block only
                later stores/sigmoids, which have a batch of slack)."""
                ov = out_d[b].rearrange("(cb c) h w -> c cb h w", cb=CB)
                # h-halved muls+stores only where the tail matters (the last
                # batch): halving costs ~0.2us of extra DVE dispatch per cb
                # but lets the final store start half a mul block earlier.
                nh = 4 if b == B_LOC - 1 else 1
                HH = H // nh
                for cb in range(CB):
                    for hh in range(nh):
                        hs = slice(hh * HH, (hh + 1) * HH)
                        xv = xtf[b][:, cb, hs]
                        if not no_mul:
                            gw_b = gw_t[:, cb, :].unsqueeze(1).broadcast_to(
                                [128, HH, W]
                            )
                            for _ in range(mul_reps):
                                nc.vector.tensor_mul(xv, xv, gw_b)
                            xp = xv.rearrange("c h (wh p) -> c h wh p", p=2)
                            g2b = gh2_t[:, cb, hs].unsqueeze(2).broadcast_to(
                                [128, HH, W // 2, 2]
                            )
                            for _ in range(mul_reps):
                                nc.vector.tensor_mul(xp, xp, g2b)
                        nc.scalar.dma_start(out=ov[:, cb, hs], in_=xv)

            # 0-depth split-gate schedule: batch b's gates are computed
            # immediately after its own reduces (gw first: its sigmoids gate
            # the first multiply), and the mul block follows directly.  The
            # DVE queue is [sm(0), M(0), sm(1), M(1), ...]; pooling (6.8us)
            # is shorter than a mul block (8.9us), so the PE stays a batch
            # ahead and the G-stalls never cascade.
            for b in range(B_LOC):
                psA, psB = pool(b)
                ys = reduces(b, psA, psB)
                gh2_t, gw_t = gates(ys)
                muls_store(b, gh2_t, gw_t)

        if unroll > 1:
            # python-unrolled repeats: sim-only stand-in for the For_i loop
            # (TimelineSim can't execute register-mode branches)
            w = load_weights()
            for _ in range(unroll):
                body(w)
        elif n_iter == 1:
            body(load_weights())
        else:
            with tc.For_i(0, n_iter, 1):
                body(load_weights())
    nc.compile()
    return nc


def get_module(n_iter: int = 1, **kwargs):
    key = (n_iter, tuple(sorted(kwargs.items())))
    if key not in _NC_CACHE:
        _NC_CACHE[key] = build_module(n_iter, **kwargs)
    return _NC_CACHE[key]


def make_in_maps(x, w1, b1, bn_gamma, bn_beta, bn_mean, bn_var, w2, b2, w3, b3):
    f64 = np.float64
    s_bn = (bn_gamma.astype(f64) / np.sqrt(bn_var.astype(f64) + 1e-5))
    w1p = (w1.astype(f64) * s_bn[:, None] / 64.0).astype(np.float32)  # [128, 256]
    b1c = ((b1.astype(f64) - bn_mean.astype(f64)) * s_bn + bn_beta.astype(f64)).astype(
        np.float32
    )
    consts = {
        "w1t": np.ascontiguousarray(
            w1p.T.reshape(CB, 128, 128).transpose(1, 0, 2)
        ).astype(NP_BF16),
        "b1c": np.ascontiguousarray(b1c.reshape(128, 1)),
        "w2t": np.ascontiguousarray(w2.T.reshape(128, 2, 128)).astype(NP_BF16),
        "b1c2": np.ascontiguousarray((b1c / 6.0 + 0.5).reshape(128, 1)),
        "b2c": np.ascontiguousarray(b2.reshape(2, 128).T),
        "w3t": np.ascontiguousarray(w3.T.reshape(128, 2, 128)).astype(NP_BF16),
        "b3c": np.ascontiguousarray(b3.reshape(2, 128).T),
    }
    xb = np.ascontiguousarray(x).astype(NP_BF16)
    in_maps = []
    for i in range(N_CORES):
        m = {"x": xb[i * B_LOC : (i + 1) * B_LOC]}
        m.update(consts)
        in_maps.append(m)
    return in_maps


def kernel(**inputs) -> np.ndarray:
    nc = get_module(1)
    in_maps = make_in_maps(**inputs)
    res = run_bass_kernel_spmd(nc, in_maps, core_ids=list(range(N_CORES)))
    out = np.concatenate([res.results[i]["out"] for i in range(N_CORES)], axis=0)
    return out.astype(np.float32)
